# revision 18
# baseline (speedup 1.0000x reference)
"""BandhaAttention Trainium2 kernel.

Sharding: 8 cores = 2 (batch) x 4 (head groups of 4 heads).
Per core: qkv projection for its 4 heads via split-fp8 DoubleRow matmuls
(x = x_hi + x_lo fp8 residual pair, w = w_hi + w_lo16 scaled residual;
3 products per chunk-pair at 0.5 cycles/row = 0.75x bf16 cost, accuracy
better than bf16). Scores via half-split fp8 DoubleRow: q stored as
(16*gate*q) hi/lo fp8 plane pair, k as fp8(k/16); one DoubleRow matmul
computes q_hi*k + q_lo*k at half bf16 cost. Causal mask folded into the
score PSUM accumulation as an identity x mask-matrix matmul (-1000 above
diagonal) so exp produces exact zeros and the DVE mask multiply is gone.
Attention (exp on ACT, AV with V-stationary bf16 matmuls, ones column ->
softmax sums for free), normalization via gpsimd partition_broadcast,
out-projection row-sharded, bf16 partial outputs. Host sums the 4
partial outputs per batch in f32.
"""

import os
import sys

import numpy as np

for p in ("/opt/trn_rl_repo", "/opt/trn_rl_repo/concourse"):
    if p not in sys.path and os.path.isdir(p):
        sys.path.insert(0, p)

import ml_dtypes

import concourse.bacc as bacc
import concourse.mybir as mybir
from concourse.bass_utils import run_bass_kernel_spmd
from concourse.tile import TileContext

BF16 = mybir.dt.bfloat16
F32 = mybir.dt.float32
FP8 = mybir.dt.float8e4
AF = mybir.ActivationFunctionType
DR = mybir.MatmulPerfMode.DoubleRow

T = 2048
D = 1024
HD = 64
NH_LOC = 4      # heads per core
DL = NH_LOC * HD  # 256 local qkv channels
KT = D // 128   # 8 contraction chunks
NCP = KT // 2   # 4 chunk pairs for DoubleRow
NQ = T // 512   # 4 tq chunks of 512
NTT = T // 128  # 16 tiles of 128

TALA = [5, 6, 7, 8]

LAST = None  # last BassKernelResults (for profiling from test.py)


def build_nc(reps=1):
    nc = bacc.Bacc("TRN2", target_bir_lowering=False)
    xh_d = nc.dram_tensor("xh", [D, T], FP8, kind="ExternalInput")
    xl_d = nc.dram_tensor("xl", [D, T], FP8, kind="ExternalInput")
    xs_d = nc.dram_tensor("xs", [D, T], FP8, kind="ExternalInput")
    wqkh_d = nc.dram_tensor("wqkh", [D, 2 * DL], FP8, kind="ExternalInput")
    wqkl_d = nc.dram_tensor("wqkl", [D, 2 * DL], FP8, kind="ExternalInput")
    wvh_d = nc.dram_tensor("wvh", [D, DL], FP8, kind="ExternalInput")
    wvl_d = nc.dram_tensor("wvl", [D, DL], FP8, kind="ExternalInput")
    wout_d = nc.dram_tensor("wout", [DL, D], BF16, kind="ExternalInput")
    gate_d = nc.dram_tensor("gate", [DL, T], BF16, kind="ExternalInput")
    imask_d = nc.dram_tensor("imask", [128, 256], BF16, kind="ExternalInput")
    out_d = nc.dram_tensor("out", [T, D], BF16, kind="ExternalOutput")

    with TileContext(nc) as tc:
      for rep in range(reps):
        with (
            tc.tile_pool(name=f"pers{rep}", bufs=2) as pers,
            tc.tile_pool(name=f"pc1{rep}", bufs=1) as pc1,
            tc.tile_pool(name=f"pv{rep}", bufs=NTT) as pv,
        ):
            # ---- constants ----
            imask = pc1.tile([128, 256], BF16, tag="imask", name="imask")
            nc.sync.dma_start(imask, imask_d[:, :])
            ident = imask[:, 0:128]
            mask = imask[:, 128:256]
            wout_big = pers.tile([128, 2 * D], BF16, tag="wout",
                                 name="wout_big", bufs=1)
            wout_sb = [wout_big[:, c * D:(c + 1) * D] for c in range(2)]

            # persistent products of phase 1
            # q: fp8 plane pair (hi | lo) of 16*gate*q;  k: fp8 of k/16
            q2_sb = [pers.tile([128, 2 * T], FP8, tag="qp", name="q2_sb")
                     for _ in range(2)]
            k1_sb = [pers.tile([128, T], FP8, tag="kp", name="k1_sb")
                     for _ in range(2)]
            v_all = [pv.tile([128, NH_LOC * (HD + 1)], BF16, tag="vall",
                             name="v_all") for _ in range(NTT)]
            aoT = [pers.tile([128, T], BF16, tag="aoT", name="aoT")
                   for _ in range(2)]

            # ---- phase 1 + 2 interleaved ----
            with (
                tc.tile_pool(name=f"pin{rep}", bufs=KT) as pin,
                tc.tile_pool(name=f"pg{rep}", bufs=2) as pg,
                tc.tile_pool(name=f"pexp{rep}", bufs=2) as pexp,
                tc.tile_pool(name=f"poex{rep}", bufs=3) as poex,
                tc.tile_pool(name=f"psm{rep}", bufs=2) as psm,
                tc.tile_pool(name=f"pstg{rep}", bufs=2) as pstg,
                tc.tile_pool(name=f"psq{rep}", bufs=2, space="PSUM") as psq,
                tc.tile_pool(name=f"pst{rep}", bufs=2, space="PSUM") as pst,
                tc.tile_pool(name=f"pav{rep}", bufs=2, space="PSUM") as pav,
            ):
                # persistent inputs; xt variants DMA'd in token-column chunks
                # so the first projection matmuls start early
                gate_big = pin.tile([128, 2 * T], BF16, tag="gate",
                                    name="gate_big", bufs=1)
                wqk_sb = {}
                wv_sb = {}
                for nm, dram, wt in (("h", wqkh_d, 2 * DL), ("l", wqkl_d, 2 * DL)):
                    t_ = pin.tile([128, KT * wt], FP8, tag=f"wqk{nm}",
                                  name=f"wqk{nm}", bufs=1)
                    nc.sync.dma_start(
                        t_.rearrange("p (a c) -> p a c", c=wt),
                        dram[:, :].rearrange("(a p) c -> p a c", p=128))
                    wqk_sb[nm] = t_.rearrange("p (a c) -> p a c", c=wt)
                xt4 = {}
                for nm, dram in (("h", xh_d), ("l", xl_d), ("s", xs_d)):
                    t_ = pin.tile([128, KT * T], FP8, tag=f"xt{nm}",
                                  name=f"xt{nm}", bufs=1)
                    # chunk-major: token-chunk n outer, k-chunk a, 512 tokens
                    xt4[nm] = t_.rearrange("p (n a t) -> p n a t", n=4, t=512)

                x_drams = {"h": xh_d, "l": xl_d, "s": xs_d}

                def load_x_chunk(n, nm):
                    # 512-token column chunk of one x variant; dst region is
                    # contiguous in SBUF (chunk-major layout)
                    nc.sync.dma_start(
                        xt4[nm][:, n],
                        x_drams[nm][:, :].rearrange("(a p) t -> p a t", p=128)[
                            :, :, n * 512:(n + 1) * 512])

                # q-critical data first: hi/lo chunks 0-1 + gate, then the
                # x_s correction chunks, then the rest
                load_x_chunk(0, "h")
                load_x_chunk(0, "l")
                load_x_chunk(1, "h")
                load_x_chunk(1, "l")
                nc.sync.dma_start(
                    gate_big.rearrange("p (a t) -> p a t", t=T),
                    gate_d[:, :].rearrange("(a p) t -> p a t", p=128))
                load_x_chunk(0, "s")
                load_x_chunk(1, "s")
                for nm, dram, wt in (("h", wvh_d, DL), ("l", wvl_d, DL)):
                    t_ = pin.tile([128, KT * wt], FP8, tag=f"wv{nm}",
                                  name=f"wv{nm}", bufs=1)
                    nc.sync.dma_start(
                        t_.rearrange("p (a c) -> p a c", c=wt),
                        dram[:, :].rearrange("(a p) c -> p a c", p=128))
                    wv_sb[nm] = t_.rearrange("p (a c) -> p a c", c=wt)
                for n in range(2, 4):
                    for nm in ("h", "l", "s"):
                        load_x_chunk(n, nm)
                nc.sync.dma_start(
                    wout_big.rearrange("p (a c) -> p a c", c=D),
                    wout_d[:, :].rearrange("(a p) c -> p a c", p=128))

                gate_sb = [gate_big[:, c * T:(c + 1) * T] for c in range(2)]

                def qkv_mms(ps, w_ap, x_ap, w_is_lhs):
                    """12 DoubleRow product matmuls accumulating into ps.
                    Products: x_hi*w_hi + x_lo*w_hi + x_s*w_lo (w_lo is
                    16x-scaled, x_s is x/16)."""
                    prods = [("h", "h"), ("h", "l"), ("l", "s")]
                    n_ = 0
                    for wv_, xv_ in prods:
                        for cp in range(NCP):
                            wa = w_ap(wv_, cp)
                            xa = x_ap(xv_, cp)
                            lhsT, rhs = (wa, xa) if w_is_lhs else (xa, wa)
                            nc.tensor.matmul(
                                ps, lhsT=lhsT, rhs=rhs,
                                start=(n_ == 0), stop=(n_ == 11),
                                perf_mode=DR,
                            )
                            n_ += 1

                def do_qk(m, n0, n1):  # m-tile of q/k, tq chunks [n0,n1)
                    for n in range(n0, n1):
                        ps = psq.tile([128, 512], F32, tag="psq", name="ps_qk")
                        qkv_mms(
                            ps,
                            lambda wv_, cp: wqk_sb[wv_][
                                :, 2 * cp:2 * cp + 2, m * 128:(m + 1) * 128],
                            lambda xv_, cp: xt4[xv_][
                                :, n, 2 * cp:2 * cp + 2, :],
                            w_is_lhs=True,
                        )
                        if m < 2:  # q: gate16-mul, then split to fp8 hi/lo
                            g = pg.tile([128, 512], F32, tag="g16", name="g16")
                            nc.vector.tensor_mul(
                                g, ps, gate_sb[m][:, n * 512:(n + 1) * 512])
                            q2v = q2_sb[m].rearrange("p (o t) -> p o t", t=T)
                            hi = q2v[:, 0, n * 512:(n + 1) * 512]
                            lo = q2v[:, 1, n * 512:(n + 1) * 512]
                            nc.gpsimd.tensor_copy(hi, g)
                            nc.gpsimd.tensor_sub(lo, g, hi)
                        else:      # k: scale 1/16 into fp8
                            nc.vector.tensor_scalar_mul(
                                k1_sb[m - 2][:, n * 512:(n + 1) * 512], ps,
                                1.0 / 16.0)

                def do_v(t):  # v natural t-tile (128, 256) -> v_all
                    ps = psq.tile([128, DL], F32, tag="psq", name="ps_v")
                    qkv_mms(
                        ps,
                        lambda wv_, cp: wv_sb[wv_][:, 2 * cp:2 * cp + 2, :],
                        lambda xv_, cp: xt4[xv_][
                            :, t // 4, 2 * cp:2 * cp + 2,
                            (t % 4) * 128:(t % 4) * 128 + 128],
                        w_is_lhs=False,
                    )
                    src = ps.rearrange("p (h c) -> p h c", c=HD)
                    dst = v_all[t].rearrange("p (h c) -> p h c", c=HD + 1)
                    nc.vector.tensor_copy(dst[:, :, 0:HD], src)
                    nc.vector.memset(dst[:, :, HD:HD + 1], 1.0)

                expt = {}  # (h, i) -> tile covering tq cols [128*i, T)

                def score_mm(st, p, hh, i, c0, a, w, start, stop):
                    """DoubleRow score matmul: (q_hi + q_lo) * k for queries
                    [a, a+w) into the piece tile st (piece base 128*i+c0)."""
                    kk = k1_sb[p][hh * 64:hh * 64 + 64,
                                  i * 128:(i + 1) * 128]
                    kk2 = kk.rearrange("p (o t) -> p o t", o=1).broadcast_to(
                        [64, 2, 128])
                    qq = q2_sb[p].rearrange("p (o t) -> p o t", t=T)[
                        hh * 64:hh * 64 + 64, :, a:a + w]
                    c = a - 128 * i - c0
                    nc.tensor.matmul(
                        st[:, c:c + w], lhsT=kk2, rhs=qq,
                        start=start, stop=stop, perf_mode=DR)

                def do_st_piece(p, i, c0):
                    w_i = T - 128 * i
                    if c0 == 0:
                        e0 = pexp.tile([128, w_i], BF16, tag=f"e{i}", name="e0")
                        e1 = pexp.tile([128, w_i], BF16, tag=f"e{i}", name="e1")
                        expt[(2 * p, i)] = e0
                        expt[(2 * p + 1, i)] = e1
                    w = min(1024, w_i - c0)
                    sts = []
                    for hh in range(2):
                        st = pst.tile([128, 1024], F32, tag="st", name="st_ps")
                        if c0 == 0:
                            # causal mask for the diagonal 128 cols via
                            # identity @ mask matmul, then scores accumulate
                            nc.tensor.matmul(
                                st[:, 0:128], lhsT=ident, rhs=mask,
                                start=True, stop=False)
                            score_mm(st, p, hh, i, 0, 128 * i, 128,
                                     start=False, stop=True)
                            if w > 128:
                                score_mm(st, p, hh, i, 0, 128 * i + 128,
                                         min(384, w - 128),
                                         start=True, stop=True)
                            if w > 512:
                                score_mm(st, p, hh, i, 0, 128 * i + 512,
                                         w - 512, start=True, stop=True)
                        else:
                            for nn in range(0, w, 512):
                                wn = min(512, w - nn)
                                score_mm(st, p, hh, i, c0,
                                         128 * i + c0 + nn, wn,
                                         start=True, stop=True)
                        sts.append(st)
                    for hh, st in enumerate(sts):
                        e = expt[(2 * p + hh, i)]
                        nc.scalar.activation(
                            e[:, c0:c0 + w], st[:, 0:w], AF.Exp, scale=0.125)

                av_tiles = {}

                def do_av_part(p, hh, j, i0, i1):
                    h = 2 * p + hh
                    last_i = 4 * j + 3
                    if i0 == 0:
                        # pair-1 final chunk: use the idle qkv psum banks so
                        # its early matmuls can run as in-loop filler without
                        # competing with the projection accumulators
                        pool, tg = (psq, "psq") if j == 3 else (pav, "av")
                        av_tiles[(p, hh)] = pool.tile([128, 512], F32,
                                                      tag=tg, name="av_ps")
                    av = av_tiles[(p, hh)]
                    for i in range(i0, i1):
                        off = 512 * j - 128 * i
                        r = max(0, -off)  # 128*(i%4) on diagonal tiles
                        nc.tensor.matmul(
                            av[0:HD + 1, r:512],
                            lhsT=v_all[i][:, hh * 65 + p * 130:
                                          hh * 65 + p * 130 + 65],
                            rhs=expt[(h, i)][:, off + r:off + 512],
                            start=(i == 0), stop=(i == last_i),
                        )
                    if i1 != last_i + 1:
                        return
                    rc = psm.tile([1, 512], F32, tag="rc", name="rc_sb")
                    nc.vector.reciprocal(rc, av[HD:HD + 1, :])
                    bc = psm.tile([64, 512], F32, tag="bc", name="bc_sb")
                    nc.gpsimd.partition_broadcast(bc, rc)
                    nc.vector.tensor_mul(
                        aoT[p][hh * 64:hh * 64 + 64, j * 512:(j + 1) * 512],
                        av[0:HD, :], bc)

                stg_tiles = {}

                def do_proj(t, n):
                    po = pav.tile([128, 512], F32, tag="av", name="po_ps")
                    for c in range(2):
                        nc.tensor.matmul(
                            po,
                            lhsT=aoT[c][:, t * 128:(t + 1) * 128],
                            rhs=wout_sb[c][:, n * 512:(n + 1) * 512],
                            start=(c == 0), stop=(c == 1),
                        )
                    if n == 0:
                        stg_tiles[t] = pstg.tile([128, D], BF16, tag="stg",
                                                 name="stg_sb")
                    stg = stg_tiles[t]
                    nc.vector.tensor_copy(stg[:, n * 512:(n + 1) * 512], po)
                    if n == 1:
                        nc.sync.dma_start(out_d[t * 128:(t + 1) * 128, :], stg)

                def do_proj_tail(t):
                    po = pst.tile([128, 1024], F32, tag="st", name="po_tail")
                    for n in range(2):
                        for c in range(2):
                            nc.tensor.matmul(
                                po[:, n * 512:(n + 1) * 512],
                                lhsT=aoT[c][:, t * 128:(t + 1) * 128],
                                rhs=wout_sb[c][:, n * 512:(n + 1) * 512],
                                start=(c == 0), stop=(c == 1),
                            )
                    stg = pstg.tile([128, D], BF16, tag="stg", name="stg_sb")
                    nc.scalar.copy(stg, po)
                    nc.sync.dma_start(out_d[t * 128:(t + 1) * 128, :], stg)

                def qk_unit(m, n):
                    return lambda: do_qk(m, n, n + 1)

                def v_unit(t):
                    return lambda: do_v(t)

                def st_units(p, j):
                    # c0=0 pieces first: they need only the first half of q
                    units = []
                    for c0 in (0, 1024):
                        for i in range(4 * j, 4 * j + 4):
                            w_i = T - 128 * i
                            if c0 < w_i:
                                units.append(
                                    (lambda p=p, i=i, c0=c0:
                                     do_st_piece(p, i, c0)))
                    return units

                def av_units(p, j):
                    units = []
                    last_i = 4 * j + 3
                    for hh in range(2):
                        for i0 in range(0, last_i + 1, 4):
                            i1 = min(i0 + 4, last_i + 1)
                            units.append(
                                (lambda p=p, hh=hh, j=j, i0=i0, i1=i1:
                                 do_av_part(p, hh, j, i0, i1)))
                    return units

                def proj_units(j):
                    return [(lambda t=t, n=n: do_proj(t, n))
                            for t in range(4 * j, 4 * j + 4) for n in range(2)]

                def winterleave(primary, filler):
                    # cost-weighted interleave: (cost_ns, fn) pairs
                    tp = sum(c for c, _ in primary) or 1
                    tf = sum(c for c, _ in filler)
                    fi = 0
                    ap = 0.0
                    af = 0.0
                    for c, fn in primary:
                        fn()
                        ap += c
                        while fi < len(filler) and af < tf * ap / tp - 1e-6:
                            fc, ff = filler[fi]
                            ff()
                            af += fc
                            fi += 1
                    for _, ff in filler[fi:]:
                        ff()

                def st_costed(p, j):
                    # pacing weight = ACT cost of the piece's exp
                    units = []
                    for c0 in (0, 1024):
                        for i in range(4 * j, 4 * j + 4):
                            w_i = T - 128 * i
                            if c0 < w_i:
                                w = min(1024, w_i - c0)
                                units.append(
                                    (0.83 * w + 190,
                                     lambda p=p, i=i, c0=c0:
                                     do_st_piece(p, i, c0)))
                    return units

                QKC = 1280.0
                VC = 640.0
                AVC = 854.0
                PRC = 700.0

                def qk_c(m, n):
                    return (QKC, qk_unit(m, n))

                def v_c(t):
                    return (VC, v_unit(t))

                def av_c(p, j):
                    return [(AVC, u) for u in av_units(p, j)]

                def proj_c(j):
                    return [(PRC, u) for u in proj_units(j)]

                # --- prologue: j=0 of pair 0, sequenced around DMA chunk
                # arrival.  piece (i, c0) reads q columns
                # [128i+c0, 128i+c0+1024), so q chunk 2 must precede piece
                # (1, 0) and chunk 3 must precede piece (0, 1024).
                do_qk(0, 0, 2)
                do_qk(2, 0, 1)
                do_st_piece(0, 0, 0)
                do_qk(0, 2, 3)
                do_st_piece(0, 1, 0)
                do_qk(0, 3, 4)
                do_st_piece(0, 2, 0)
                do_qk(2, 1, 2)
                do_st_piece(0, 3, 0)
                do_v(0)
                do_st_piece(0, 0, 1024)
                do_qk(1, 0, 1)
                do_st_piece(0, 1, 1024)
                do_v(1)
                do_st_piece(0, 2, 1024)
                do_qk(1, 1, 2)
                do_st_piece(0, 3, 1024)
                do_v(2)
                do_v(3)
                # --- main stream: remaining score blocks with PE fillers in
                # dependency order, weighted by cost so ACT stays fed
                av03 = av_units(0, 3)
                av3 = av_units(1, 3)
                blocks = [
                    (st_costed(0, 1),
                     [qk_c(2, 2)] + av_c(0, 0) + [v_c(4), v_c(5), v_c(6),
                      qk_c(1, 2), v_c(7), v_c(8), qk_c(1, 3), v_c(9)]),
                    (st_costed(0, 2),
                     [qk_c(2, 3)] + av_c(0, 1) + [v_c(10), v_c(11), v_c(12),
                      qk_c(3, 0), v_c(13), qk_c(3, 1), v_c(14), qk_c(3, 2),
                      v_c(15), qk_c(3, 3)]),
                    (st_costed(0, 3), av_c(0, 2)),
                    (st_costed(1, 0), [(AVC, u) for u in av03]),
                    (st_costed(1, 1), av_c(1, 0)),
                    (st_costed(1, 2), av_c(1, 1) + proj_c(0)),
                    (st_costed(1, 3),
                     av_c(1, 2)
                     + [(AVC, u) for idx, u in enumerate(av3) if idx % 4 != 3]
                     + proj_c(1) + proj_c(2)),
                ]
                for primary, filler in blocks:
                    winterleave(primary, filler)
                # --- tail: last AV chunks, final normalize, projection of
                # the last query block (wide psum + ACT copy, DMA out)
                for u in [u for idx, u in enumerate(av3) if idx % 4 == 3]:
                    u()
                for t in range(12, 16):
                    do_proj_tail(t)
    nc.compile()
    return nc


def _prep_inputs(x, w_qkv, w_out, bandha_gate):
    bf = ml_dtypes.bfloat16
    f8 = ml_dtypes.float8_e4m3
    t = np.arange(T)
    gate_full = np.empty((16, T), np.float64)
    for h in range(16):
        cyc = TALA[h % len(TALA)]
        gate_full[h] = 1.0 / (1.0 + np.exp(-bandha_gate[h, t % cyc].astype(np.float64)))
    gate16 = 16.0 * gate_full
    # identity | mask (-1000 above diagonal in (key, query) indexing)
    ident = np.eye(128, dtype=np.float64)
    mask = np.where(np.arange(128)[None, :] >= np.arange(128)[:, None],
                    0.0, -1000.0)
    imask = np.concatenate([ident, mask], axis=1).astype(bf)

    def split8(a, s):
        hi = a.astype(f8)
        lo = ((a - hi.astype(np.float64)) * s).astype(f8)
        return hi, lo

    x64 = x.astype(np.float64)
    w64 = w_qkv.astype(np.float64)
    in_maps = []
    for c in range(8):
        b, g = c // 4, c % 4
        xt = np.ascontiguousarray(x64[b].T)
        xh, xl = split8(xt, 1.0)
        xs = (xt / 16.0).astype(f8)
        wqk = np.concatenate(
            [w64[:, g * DL:(g + 1) * DL],
             w64[:, D + g * DL:D + (g + 1) * DL]], axis=1)
        wqkh, wqkl = split8(wqk, 16.0)
        wv = w64[:, 2 * D + g * DL:2 * D + (g + 1) * DL]
        wvh, wvl = split8(wv, 16.0)
        wout = np.ascontiguousarray(w_out[g * DL:(g + 1) * DL, :]).astype(bf)
        gb = np.repeat(gate16[4 * g:4 * g + 4], HD, axis=0).astype(bf)
        in_maps.append({
            "xh": np.ascontiguousarray(xh), "xl": np.ascontiguousarray(xl),
            "xs": np.ascontiguousarray(xs),
            "wqkh": np.ascontiguousarray(wqkh),
            "wqkl": np.ascontiguousarray(wqkl),
            "wvh": np.ascontiguousarray(wvh), "wvl": np.ascontiguousarray(wvl),
            "wout": wout, "gate": np.ascontiguousarray(gb), "imask": imask})
    return in_maps


def kernel(**inputs):
    global LAST
    x = np.asarray(inputs["x"], np.float32)
    w_qkv = np.asarray(inputs["w_qkv"], np.float32)
    w_out = np.asarray(inputs["w_out"], np.float32)
    bandha_gate = np.asarray(inputs["bandha_gate"], np.float32)

    in_maps = _prep_inputs(x, w_qkv, w_out, bandha_gate)
    nc = build_nc()
    res = run_bass_kernel_spmd(
        nc, in_maps, core_ids=list(range(8)),
        trace=os.environ.get("BANDHA_TRACE") == "1",
    )
    LAST = res
    outs = [r["out"] for r in res.results]
    full = np.empty((2, T, D), np.float32)
    for b in range(2):
        full[b] = (outs[4 * b].astype(np.float32)
                   + outs[4 * b + 1].astype(np.float32)
                   + outs[4 * b + 2].astype(np.float32)
                   + outs[4 * b + 3].astype(np.float32))
    return full


# revision 19
# speedup vs baseline: 1.0575x; 1.0575x over previous
"""BandhaAttention Trainium2 kernel.

Sharding: 8 cores = 2 (batch) x 4 (head groups of 4 heads).
Per core: qkv projection for its 4 heads via split-fp8 DoubleRow matmuls
(x = x_hi + x_lo fp8 residual pair, w = w_hi + w_lo16 scaled residual;
3 products per chunk-pair at 0.5 cycles/row = 0.75x bf16 cost, accuracy
better than bf16). Scores via half-split fp8 DoubleRow: q stored as
(16*gate*q) hi/lo fp8 plane pair, k as fp8(k/16); one DoubleRow matmul
computes q_hi*k + q_lo*k at half bf16 cost. Causal mask folded into the
score PSUM accumulation as an identity x mask-matrix matmul (-1000 above
diagonal) so exp produces exact zeros and the DVE mask multiply is gone.
Attention (exp on ACT, AV with V-stationary bf16 matmuls, ones column ->
softmax sums for free), normalization via gpsimd partition_broadcast,
out-projection row-sharded, bf16 partial outputs. Host sums the 4
partial outputs per batch in f32.
"""

import os
import sys

import numpy as np

for p in ("/opt/trn_rl_repo", "/opt/trn_rl_repo/concourse"):
    if p not in sys.path and os.path.isdir(p):
        sys.path.insert(0, p)

import ml_dtypes

import concourse.bacc as bacc
import concourse.mybir as mybir
from concourse.bass_utils import run_bass_kernel_spmd
from concourse.tile import TileContext

BF16 = mybir.dt.bfloat16
F32 = mybir.dt.float32
FP8 = mybir.dt.float8e4
AF = mybir.ActivationFunctionType
DR = mybir.MatmulPerfMode.DoubleRow

T = 2048
D = 1024
HD = 64
NH_LOC = 4      # heads per core
DL = NH_LOC * HD  # 256 local qkv channels
KT = D // 128   # 8 contraction chunks
NCP = KT // 2   # 4 chunk pairs for DoubleRow
NQ = T // 512   # 4 tq chunks of 512
NTT = T // 128  # 16 tiles of 128

TALA = [5, 6, 7, 8]

LAST = None  # last BassKernelResults (for profiling from test.py)


def build_nc(reps=1):
    nc = bacc.Bacc("TRN2", target_bir_lowering=False)
    xh_d = nc.dram_tensor("xh", [D, T], FP8, kind="ExternalInput")
    xl_d = nc.dram_tensor("xl", [D, T], FP8, kind="ExternalInput")
    xs_d = nc.dram_tensor("xs", [D, T], FP8, kind="ExternalInput")
    wqkh_d = nc.dram_tensor("wqkh", [D, 2 * DL], FP8, kind="ExternalInput")
    wqkl_d = nc.dram_tensor("wqkl", [D, 2 * DL], FP8, kind="ExternalInput")
    wvh_d = nc.dram_tensor("wvh", [D, DL], FP8, kind="ExternalInput")
    wvl_d = nc.dram_tensor("wvl", [D, DL], FP8, kind="ExternalInput")
    wout_d = nc.dram_tensor("wout", [DL, D], BF16, kind="ExternalInput")
    gate_d = nc.dram_tensor("gate", [DL, T], BF16, kind="ExternalInput")
    imask_d = nc.dram_tensor("imask", [128, 256], BF16, kind="ExternalInput")
    out_d = nc.dram_tensor("out", [T, D], BF16, kind="ExternalOutput")

    with TileContext(nc) as tc:
      for rep in range(reps):
        with (
            tc.tile_pool(name=f"pers{rep}", bufs=2) as pers,
            tc.tile_pool(name=f"pc1{rep}", bufs=1) as pc1,
            tc.tile_pool(name=f"pv{rep}", bufs=NTT) as pv,
        ):
            # ---- constants ----
            imask = pc1.tile([128, 256], BF16, tag="imask", name="imask")
            nc.sync.dma_start(imask, imask_d[:, :])
            ident = imask[:, 0:128]
            mask = imask[:, 128:256]
            wout_big = pers.tile([128, 2 * D], BF16, tag="wout",
                                 name="wout_big", bufs=1)
            wout_sb = [wout_big[:, c * D:(c + 1) * D] for c in range(2)]

            # persistent products of phase 1
            # q: fp8 plane pair (hi | lo) of 16*gate*q;  k: fp8 of k/16
            q2_sb = [pers.tile([128, 2 * T], FP8, tag="qp", name="q2_sb")
                     for _ in range(2)]
            k1_sb = [pers.tile([128, T], FP8, tag="kp", name="k1_sb")
                     for _ in range(2)]
            v_all = [pv.tile([128, NH_LOC * (HD + 1)], BF16, tag="vall",
                             name="v_all") for _ in range(NTT)]
            aoT = [pers.tile([128, T], BF16, tag="aoT", name="aoT")
                   for _ in range(2)]

            # ---- phase 1 + 2 interleaved ----
            with (
                tc.tile_pool(name=f"pin{rep}", bufs=KT) as pin,
                tc.tile_pool(name=f"pg{rep}", bufs=2) as pg,
                tc.tile_pool(name=f"pexp{rep}", bufs=2) as pexp,
                tc.tile_pool(name=f"poex{rep}", bufs=3) as poex,
                tc.tile_pool(name=f"psm{rep}", bufs=2) as psm,
                tc.tile_pool(name=f"pstg{rep}", bufs=2) as pstg,
                tc.tile_pool(name=f"psq{rep}", bufs=2, space="PSUM") as psq,
                tc.tile_pool(name=f"pst{rep}", bufs=2, space="PSUM") as pst,
                tc.tile_pool(name=f"pav{rep}", bufs=2, space="PSUM") as pav,
            ):
                # persistent inputs; xt variants DMA'd in token-column chunks
                # so the first projection matmuls start early
                gate_big = pin.tile([128, 2 * T], BF16, tag="gate",
                                    name="gate_big", bufs=1)
                wqk_sb = {}
                wv_sb = {}
                for nm, dram, wt in (("h", wqkh_d, 2 * DL), ("l", wqkl_d, 2 * DL)):
                    t_ = pin.tile([128, KT * wt], FP8, tag=f"wqk{nm}",
                                  name=f"wqk{nm}", bufs=1)
                    nc.sync.dma_start(
                        t_.rearrange("p (a c) -> p a c", c=wt),
                        dram[:, :].rearrange("(a p) c -> p a c", p=128))
                    wqk_sb[nm] = t_.rearrange("p (a c) -> p a c", c=wt)
                xt4 = {}
                for nm, dram in (("h", xh_d), ("l", xl_d), ("s", xs_d)):
                    t_ = pin.tile([128, KT * T], FP8, tag=f"xt{nm}",
                                  name=f"xt{nm}", bufs=1)
                    # chunk-major: token-chunk n outer, k-chunk a, 512 tokens
                    xt4[nm] = t_.rearrange("p (n a t) -> p n a t", n=4, t=512)

                x_drams = {"h": xh_d, "l": xl_d, "s": xs_d}

                def load_x_chunk(n, nm):
                    # 512-token column chunk of one x variant; dst region is
                    # contiguous in SBUF (chunk-major layout)
                    nc.sync.dma_start(
                        xt4[nm][:, n],
                        x_drams[nm][:, :].rearrange("(a p) t -> p a t", p=128)[
                            :, :, n * 512:(n + 1) * 512])

                # q-critical data first: hi/lo chunks 0-1 + gate, then the
                # x_s correction chunks, then the rest
                load_x_chunk(0, "h")
                load_x_chunk(0, "l")
                load_x_chunk(1, "h")
                load_x_chunk(1, "l")
                load_x_chunk(0, "s")
                load_x_chunk(1, "s")
                gate_r = gate_d[:, :].rearrange("(a p) t -> p a t", p=128)
                gate_bv = gate_big.rearrange("p (a t) -> p a t", t=T)
                nc.sync.dma_start(gate_bv[:, 0:1, :], gate_r[:, 0:1, :])
                for nm, dram, wt in (("h", wvh_d, DL), ("l", wvl_d, DL)):
                    t_ = pin.tile([128, KT * wt], FP8, tag=f"wv{nm}",
                                  name=f"wv{nm}", bufs=1)
                    nc.sync.dma_start(
                        t_.rearrange("p (a c) -> p a c", c=wt),
                        dram[:, :].rearrange("(a p) c -> p a c", p=128))
                    wv_sb[nm] = t_.rearrange("p (a c) -> p a c", c=wt)
                nc.sync.dma_start(gate_bv[:, 1:2, :], gate_r[:, 1:2, :])
                for n in range(2, 4):
                    for nm in ("h", "l", "s"):
                        load_x_chunk(n, nm)
                nc.sync.dma_start(
                    wout_big.rearrange("p (a c) -> p a c", c=D),
                    wout_d[:, :].rearrange("(a p) c -> p a c", p=128))

                gate_sb = [gate_big[:, c * T:(c + 1) * T] for c in range(2)]

                def qkv_mms(ps, w_ap, x_ap, w_is_lhs):
                    """12 DoubleRow product matmuls accumulating into ps.
                    Products: x_hi*w_hi + x_lo*w_hi + x_s*w_lo (w_lo is
                    16x-scaled, x_s is x/16)."""
                    prods = [("h", "h"), ("h", "l"), ("l", "s")]
                    n_ = 0
                    for wv_, xv_ in prods:
                        for cp in range(NCP):
                            wa = w_ap(wv_, cp)
                            xa = x_ap(xv_, cp)
                            lhsT, rhs = (wa, xa) if w_is_lhs else (xa, wa)
                            nc.tensor.matmul(
                                ps, lhsT=lhsT, rhs=rhs,
                                start=(n_ == 0), stop=(n_ == 11),
                                perf_mode=DR,
                            )
                            n_ += 1

                def do_qk(m, n0, n1):  # m-tile of q/k, tq chunks [n0,n1)
                    for n in range(n0, n1):
                        ps = psq.tile([128, 512], F32, tag="psq", name="ps_qk")
                        qkv_mms(
                            ps,
                            lambda wv_, cp: wqk_sb[wv_][
                                :, 2 * cp:2 * cp + 2, m * 128:(m + 1) * 128],
                            lambda xv_, cp: xt4[xv_][
                                :, n, 2 * cp:2 * cp + 2, :],
                            w_is_lhs=True,
                        )
                        if m < 2:  # q: gate16-mul, then split to fp8 hi/lo
                            g = pg.tile([128, 512], F32, tag="g16", name="g16")
                            nc.vector.tensor_mul(
                                g, ps, gate_sb[m][:, n * 512:(n + 1) * 512])
                            q2v = q2_sb[m].rearrange("p (o t) -> p o t", t=T)
                            hi = q2v[:, 0, n * 512:(n + 1) * 512]
                            lo = q2v[:, 1, n * 512:(n + 1) * 512]
                            nc.gpsimd.tensor_copy(hi, g)
                            nc.gpsimd.tensor_sub(lo, g, hi)
                        else:      # k: scale 1/16 into fp8
                            nc.vector.tensor_scalar_mul(
                                k1_sb[m - 2][:, n * 512:(n + 1) * 512], ps,
                                1.0 / 16.0)

                def do_v(t):  # v natural t-tile (128, 256) -> v_all
                    ps = psq.tile([128, DL], F32, tag="psq", name="ps_v")
                    qkv_mms(
                        ps,
                        lambda wv_, cp: wv_sb[wv_][:, 2 * cp:2 * cp + 2, :],
                        lambda xv_, cp: xt4[xv_][
                            :, t // 4, 2 * cp:2 * cp + 2,
                            (t % 4) * 128:(t % 4) * 128 + 128],
                        w_is_lhs=False,
                    )
                    src = ps.rearrange("p (h c) -> p h c", c=HD)
                    dst = v_all[t].rearrange("p (h c) -> p h c", c=HD + 1)
                    nc.vector.tensor_copy(dst[:, :, 0:HD], src)
                    nc.vector.memset(dst[:, :, HD:HD + 1], 1.0)

                expt = {}  # (h, i) -> tile covering tq cols [128*i, T)

                def score_mm(st, p, hh, i, c0, a, w, start, stop):
                    """DoubleRow score matmul: (q_hi + q_lo) * k for queries
                    [a, a+w) into the piece tile st (piece base 128*i+c0)."""
                    kk = k1_sb[p][hh * 64:hh * 64 + 64,
                                  i * 128:(i + 1) * 128]
                    kk2 = kk.rearrange("p (o t) -> p o t", o=1).broadcast_to(
                        [64, 2, 128])
                    qq = q2_sb[p].rearrange("p (o t) -> p o t", t=T)[
                        hh * 64:hh * 64 + 64, :, a:a + w]
                    c = a - 128 * i - c0
                    nc.tensor.matmul(
                        st[:, c:c + w], lhsT=kk2, rhs=qq,
                        start=start, stop=stop, perf_mode=DR)

                def do_st_piece(p, i, c0):
                    w_i = T - 128 * i
                    if c0 == 0:
                        e0 = pexp.tile([128, w_i], BF16, tag=f"e{i}", name="e0")
                        e1 = pexp.tile([128, w_i], BF16, tag=f"e{i}", name="e1")
                        expt[(2 * p, i)] = e0
                        expt[(2 * p + 1, i)] = e1
                    w = min(1024, w_i - c0)
                    sts = []
                    for hh in range(2):
                        st = pst.tile([128, 1024], F32, tag="st", name="st_ps")
                        if c0 == 0:
                            # causal mask for the diagonal 128 cols via
                            # identity @ mask matmul, then scores accumulate
                            nc.tensor.matmul(
                                st[:, 0:128], lhsT=ident, rhs=mask,
                                start=True, stop=False)
                            score_mm(st, p, hh, i, 0, 128 * i, 128,
                                     start=False, stop=True)
                            if w > 128:
                                score_mm(st, p, hh, i, 0, 128 * i + 128,
                                         min(384, w - 128),
                                         start=True, stop=True)
                            if w > 512:
                                score_mm(st, p, hh, i, 0, 128 * i + 512,
                                         w - 512, start=True, stop=True)
                        else:
                            for nn in range(0, w, 512):
                                wn = min(512, w - nn)
                                score_mm(st, p, hh, i, c0,
                                         128 * i + c0 + nn, wn,
                                         start=True, stop=True)
                        sts.append(st)
                    for hh, st in enumerate(sts):
                        e = expt[(2 * p + hh, i)]
                        nc.scalar.activation(
                            e[:, c0:c0 + w], st[:, 0:w], AF.Exp, scale=0.125)

                av_tiles = {}

                def do_av_part(p, hh, j, i0, i1):
                    h = 2 * p + hh
                    last_i = 4 * j + 3
                    if i0 == 0:
                        # pair-1 final chunk: use the idle qkv psum banks so
                        # its early matmuls can run as in-loop filler without
                        # competing with the projection accumulators
                        pool, tg = (psq, "psq") if j == 3 else (pav, "av")
                        av_tiles[(p, hh)] = pool.tile([128, 512], F32,
                                                      tag=tg, name="av_ps")
                    av = av_tiles[(p, hh)]
                    for i in range(i0, i1):
                        off = 512 * j - 128 * i
                        r = max(0, -off)  # 128*(i%4) on diagonal tiles
                        nc.tensor.matmul(
                            av[0:HD + 1, r:512],
                            lhsT=v_all[i][:, hh * 65 + p * 130:
                                          hh * 65 + p * 130 + 65],
                            rhs=expt[(h, i)][:, off + r:off + 512],
                            start=(i == 0), stop=(i == last_i),
                        )
                    if i1 != last_i + 1:
                        return
                    rc = psm.tile([1, 512], F32, tag="rc", name="rc_sb")
                    nc.vector.reciprocal(rc, av[HD:HD + 1, :])
                    bc = psm.tile([64, 512], F32, tag="bc", name="bc_sb")
                    nc.gpsimd.partition_broadcast(bc, rc)
                    nc.vector.tensor_mul(
                        aoT[p][hh * 64:hh * 64 + 64, j * 512:(j + 1) * 512],
                        av[0:HD, :], bc)

                stg_tiles = {}

                def do_proj(t, n):
                    po = pav.tile([128, 512], F32, tag="av", name="po_ps")
                    for c in range(2):
                        nc.tensor.matmul(
                            po,
                            lhsT=aoT[c][:, t * 128:(t + 1) * 128],
                            rhs=wout_sb[c][:, n * 512:(n + 1) * 512],
                            start=(c == 0), stop=(c == 1),
                        )
                    if n == 0:
                        stg_tiles[t] = pstg.tile([128, D], BF16, tag="stg",
                                                 name="stg_sb")
                    stg = stg_tiles[t]
                    nc.vector.tensor_copy(stg[:, n * 512:(n + 1) * 512], po)
                    if n == 1:
                        nc.sync.dma_start(out_d[t * 128:(t + 1) * 128, :], stg)

                def do_proj_tail(t):
                    po = pst.tile([128, 1024], F32, tag="st", name="po_tail")
                    for n in range(2):
                        for c in range(2):
                            nc.tensor.matmul(
                                po[:, n * 512:(n + 1) * 512],
                                lhsT=aoT[c][:, t * 128:(t + 1) * 128],
                                rhs=wout_sb[c][:, n * 512:(n + 1) * 512],
                                start=(c == 0), stop=(c == 1),
                            )
                    stg = pstg.tile([128, D], BF16, tag="stg", name="stg_sb")
                    nc.scalar.copy(stg, po)
                    nc.sync.dma_start(out_d[t * 128:(t + 1) * 128, :], stg)

                def qk_unit(m, n):
                    return lambda: do_qk(m, n, n + 1)

                def v_unit(t):
                    return lambda: do_v(t)

                def st_units(p, j):
                    # c0=0 pieces first: they need only the first half of q
                    units = []
                    for c0 in (0, 1024):
                        for i in range(4 * j, 4 * j + 4):
                            w_i = T - 128 * i
                            if c0 < w_i:
                                units.append(
                                    (lambda p=p, i=i, c0=c0:
                                     do_st_piece(p, i, c0)))
                    return units

                def av_units(p, j):
                    units = []
                    last_i = 4 * j + 3
                    for hh in range(2):
                        for i0 in range(0, last_i + 1, 4):
                            i1 = min(i0 + 4, last_i + 1)
                            units.append(
                                (lambda p=p, hh=hh, j=j, i0=i0, i1=i1:
                                 do_av_part(p, hh, j, i0, i1)))
                    return units

                def proj_units(j):
                    return [(lambda t=t, n=n: do_proj(t, n))
                            for t in range(4 * j, 4 * j + 4) for n in range(2)]

                def winterleave(primary, filler):
                    # cost-weighted interleave: (cost_ns, fn) pairs
                    tp = sum(c for c, _ in primary) or 1
                    tf = sum(c for c, _ in filler)
                    fi = 0
                    ap = 0.0
                    af = 0.0
                    for c, fn in primary:
                        fn()
                        ap += c
                        while fi < len(filler) and af < tf * ap / tp - 1e-6:
                            fc, ff = filler[fi]
                            ff()
                            af += fc
                            fi += 1
                    for _, ff in filler[fi:]:
                        ff()

                def st_costed(p, j):
                    # pacing weight = ACT cost of the piece's exp
                    units = []
                    for c0 in (0, 1024):
                        for i in range(4 * j, 4 * j + 4):
                            w_i = T - 128 * i
                            if c0 < w_i:
                                w = min(1024, w_i - c0)
                                units.append(
                                    (0.83 * w + 190,
                                     lambda p=p, i=i, c0=c0:
                                     do_st_piece(p, i, c0)))
                    return units

                QKC = 1280.0
                VC = 640.0
                AVC = 854.0
                PRC = 700.0

                def qk_c(m, n):
                    return (QKC, qk_unit(m, n))

                def v_c(t):
                    return (VC, v_unit(t))

                def av_c(p, j):
                    return [(AVC, u) for u in av_units(p, j)]

                def proj_c(j):
                    return [(PRC, u) for u in proj_units(j)]

                # --- prologue: j=0 of pair 0, sequenced around DMA chunk
                # arrival.  piece (i, c0) reads q columns
                # [128i+c0, 128i+c0+1024), so q chunk 2 must precede piece
                # (1, 0) and chunk 3 must precede piece (0, 1024).
                # dummy matmuls (identity @ mask -> scratch psum) keep the
                # PE p-state ramped while input DMA streams in
                wu = pav.tile([128, 128], F32, tag="av", name="wu_ps")

                def warm(k):
                    for _ in range(k):
                        nc.tensor.matmul(wu, lhsT=ident, rhs=mask,
                                         start=True, stop=True)

                warm(110)
                do_qk(0, 0, 2)
                warm(16)
                do_qk(2, 0, 1)
                warm(16)
                do_st_piece(0, 0, 0)
                do_qk(0, 2, 3)
                warm(10)
                do_st_piece(0, 1, 0)
                do_qk(0, 3, 4)
                do_st_piece(0, 2, 0)
                do_qk(2, 1, 2)
                do_st_piece(0, 3, 0)
                do_v(0)
                do_st_piece(0, 0, 1024)
                do_qk(1, 0, 1)
                do_st_piece(0, 1, 1024)
                do_v(1)
                do_st_piece(0, 2, 1024)
                do_qk(1, 1, 2)
                do_st_piece(0, 3, 1024)
                do_v(2)
                do_v(3)
                # --- main stream: remaining score blocks with PE fillers in
                # dependency order, weighted by cost so ACT stays fed
                av03 = av_units(0, 3)
                av3 = av_units(1, 3)
                blocks = [
                    (st_costed(0, 1),
                     [qk_c(2, 2)] + av_c(0, 0) + [v_c(4), v_c(5), v_c(6),
                      qk_c(1, 2), v_c(7), v_c(8), qk_c(1, 3), v_c(9)]),
                    (st_costed(0, 2),
                     [qk_c(2, 3)] + av_c(0, 1) + [v_c(10), v_c(11), v_c(12),
                      qk_c(3, 0), v_c(13), qk_c(3, 1), v_c(14), qk_c(3, 2),
                      v_c(15), qk_c(3, 3)]),
                    (st_costed(0, 3), av_c(0, 2)),
                    (st_costed(1, 0), [(AVC, u) for u in av03]),
                    (st_costed(1, 1), av_c(1, 0) + proj_c(0)),
                    (st_costed(1, 2), av_c(1, 1) + proj_c(1)),
                    (st_costed(1, 3),
                     av_c(1, 2)
                     + [(AVC, u) for idx, u in enumerate(av3) if idx % 4 != 3]
                     + proj_c(2)),
                ]
                for primary, filler in blocks:
                    winterleave(primary, filler)
                # --- tail: last AV chunks, final normalize, projection of
                # the last query block (wide psum + ACT copy, DMA out)
                for u in [u for idx, u in enumerate(av3) if idx % 4 == 3]:
                    u()
                for t in range(12, 16):
                    do_proj_tail(t)
    nc.compile()
    return nc


def _prep_inputs(x, w_qkv, w_out, bandha_gate):
    bf = ml_dtypes.bfloat16
    f8 = ml_dtypes.float8_e4m3
    t = np.arange(T)
    gate_full = np.empty((16, T), np.float64)
    for h in range(16):
        cyc = TALA[h % len(TALA)]
        gate_full[h] = 1.0 / (1.0 + np.exp(-bandha_gate[h, t % cyc].astype(np.float64)))
    gate16 = 16.0 * gate_full
    # identity | mask (-1000 above diagonal in (key, query) indexing)
    ident = np.eye(128, dtype=np.float64)
    mask = np.where(np.arange(128)[None, :] >= np.arange(128)[:, None],
                    0.0, -1000.0)
    imask = np.concatenate([ident, mask], axis=1).astype(bf)

    def split8(a, s):
        hi = a.astype(f8)
        lo = ((a - hi.astype(np.float64)) * s).astype(f8)
        return hi, lo

    x64 = x.astype(np.float64)
    w64 = w_qkv.astype(np.float64)
    in_maps = []
    for c in range(8):
        b, g = c // 4, c % 4
        xt = np.ascontiguousarray(x64[b].T)
        xh, xl = split8(xt, 1.0)
        xs = (xt / 16.0).astype(f8)
        wqk = np.concatenate(
            [w64[:, g * DL:(g + 1) * DL],
             w64[:, D + g * DL:D + (g + 1) * DL]], axis=1)
        wqkh, wqkl = split8(wqk, 16.0)
        wv = w64[:, 2 * D + g * DL:2 * D + (g + 1) * DL]
        wvh, wvl = split8(wv, 16.0)
        wout = np.ascontiguousarray(w_out[g * DL:(g + 1) * DL, :]).astype(bf)
        gb = np.repeat(gate16[4 * g:4 * g + 4], HD, axis=0).astype(bf)
        in_maps.append({
            "xh": np.ascontiguousarray(xh), "xl": np.ascontiguousarray(xl),
            "xs": np.ascontiguousarray(xs),
            "wqkh": np.ascontiguousarray(wqkh),
            "wqkl": np.ascontiguousarray(wqkl),
            "wvh": np.ascontiguousarray(wvh), "wvl": np.ascontiguousarray(wvl),
            "wout": wout, "gate": np.ascontiguousarray(gb), "imask": imask})
    return in_maps


def kernel(**inputs):
    global LAST
    x = np.asarray(inputs["x"], np.float32)
    w_qkv = np.asarray(inputs["w_qkv"], np.float32)
    w_out = np.asarray(inputs["w_out"], np.float32)
    bandha_gate = np.asarray(inputs["bandha_gate"], np.float32)

    in_maps = _prep_inputs(x, w_qkv, w_out, bandha_gate)
    nc = build_nc()
    res = run_bass_kernel_spmd(
        nc, in_maps, core_ids=list(range(8)),
        trace=os.environ.get("BANDHA_TRACE") == "1",
    )
    LAST = res
    outs = [r["out"] for r in res.results]
    full = np.empty((2, T, D), np.float32)
    for b in range(2):
        full[b] = (outs[4 * b].astype(np.float32)
                   + outs[4 * b + 1].astype(np.float32)
                   + outs[4 * b + 2].astype(np.float32)
                   + outs[4 * b + 3].astype(np.float32))
    return full
